# revision 19
# baseline (speedup 1.0000x reference)
"""Bahdanau-attention kernel for Trainium2 (8 NeuronCores, data-parallel over batch).

reference math:
  energy = relu(concat([hidden bcast T, enc], -1) @ W.T + b)   # [B,T,D]
  scores = energy @ v                                          # [B,T]
  out    = softmax(scores, axis=T)[:, None, :]                 # [B,1,T]

Per-core kernel (4 batch elems, 8192 bt rows):
  W = [W1 | W2]; hb = (hid @ W1.T + b) * S is computed on host (tiny) and
  folded into the relu bias; W2.T ships pre-scaled (x S=8192) in fp8e4
  DoubleRow layout (the scale dodges e4m3 subnormals and is undone by
  shipping v/S). enc ships pre-transposed + pre-tiled (encT bf16, tiles 0-1
  pre-cast fp8) so the device does zero PE transposes: SWDGE cast-DMA loads
  encT straight into fp8 [128, 2, 512] quarter-tiles (8 KB-contiguous
  descriptor runs), the main matmul runs fp8 DoubleRow (K=256 per MM, 512
  MMs at the 213 ns N=512 streaming floor), relu(psum + hb*S) -> bf16 runs
  on ACT with d-tiles 3/7 on DVE (keeps PSUM banks recycling ahead of the
  PE), and the v-dot contracts d via 4-wide col-group-packed bf16 PE matmuls
  (tile_position) flushed one tile late so all relu deps are met, with
  cross-position DVE adds and fp32 softmax over T per batch elem. The last
  two bt-tiles instead chain single-position v-dots and exp() straight off
  the PSUM score row so the kernel tail is just one softmax. Junk warmup
  matmuls at t=0 cover the first DMA and hold the PE HAM clock-gate at 8/8.
"""
import numpy as np
import ml_dtypes
import concourse.mybir as mybir
import concourse.tile as tile
import concourse.bacc as bacc
from concourse import bass_utils

P = 128
B, T, D = 32, 2048, 1024
N_CORES = 8
NB = B // N_CORES            # 4 local batch elems
BT = NB * T                  # 8192 local rows
BTT = 512                    # bt-tile (columns of energy^T)
N_BT = BT // BTT             # 16 bt-tiles
DT = D // P                  # 8 d-tiles (output dim of W2)
KT = D // P                  # 8 k-subtiles (contraction over enc features)
NG = KT // 2                 # 4 DoubleRow groups (K=256 each)
W_SCALE = 8192.0             # keeps fp8(W2*S) in e4m3 normal range
BF16, F32 = mybir.dt.bfloat16, mybir.dt.float32
FP8 = mybir.dt.float8e4
RELU = mybir.ActivationFunctionType.Relu
EXP = mybir.ActivationFunctionType.Exp
DR = mybir.MatmulPerfMode.DoubleRow
ADD, MAX = mybir.AluOpType.add, mybir.AluOpType.max


def _build():
    nc = bacc.Bacc("TRN2", target_bir_lowering=False, debug=False)
    ENCT = nc.dram_tensor("enct", [N_BT, P, KT, BTT], BF16, kind="ExternalInput").ap()
    ENC0 = nc.dram_tensor("enc0", [2, P, KT, BTT], FP8, kind="ExternalInput").ap()
    W2Q = nc.dram_tensor("w2q", [P, KT * D], FP8, kind="ExternalInput").ap()
    HB = nc.dram_tensor("hb", [P, DT * NB], F32, kind="ExternalInput").ap()
    VT = nc.dram_tensor("vt", [P, DT], BF16, kind="ExternalInput").ap()
    OUT = nc.dram_tensor("out", [NB, T], F32, kind="ExternalOutput").ap()

    with tile.TileContext(nc) as tc, \
         tc.tile_pool(name="persist", bufs=1) as pp, \
         tc.tile_pool(name="enc_sb", bufs=32) as ep, \
         tc.tile_pool(name="e_sb", bufs=24) as ebp, \
         tc.tile_pool(name="ps_e", bufs=6, space="PSUM") as pep, \
         tc.tile_pool(name="ps_s", bufs=2, space="PSUM") as psp, \
         tc.tile_pool(name="sm", bufs=1) as smp:

        # ---- PE warmup: junk matmuls cover the initial DMA wait and get the
        # HAM clock gate to 8/8 before the first real matmul ----
        junk = pp.tile([P, P], BF16)
        nc.vector.memset(junk, 0.0)
        jps = pep.tile([P, P], F32, tag="e", name="junk_ps")
        for _ in range(52):
            nc.tensor.matmul(jps, junk, junk, start=True, stop=True)

        # persistent: DoubleRow-layout W2 in per-d-tile tiles (loaded in
        # the order the di-loop consumes them, so tile 0 pipelines instead of
        # stalling on one big weight transfer), fused hidden/bias, v/S
        w2d = [pp.tile([P, KT, P], FP8, name=f"w2d_{di}") for di in range(DT)]
        hb = pp.tile([P, DT * NB], F32)  # col di*NB+b = ((hid@W1.T)[b,d]+bias)*S
        vt = pp.tile([P, DT], BF16)      # col di = v[di*128 : (di+1)*128] / S
        W2R = W2Q.rearrange("p (di kj c) -> p di kj c", di=DT, kj=KT)

        def w2g(g, di):
            return w2d[di][:, 2 * g:2 * g + 2, :]

        # batch elem bi lives on partition 32*bi (compute outputs need
        # 32-aligned partition bases)
        scores = pp.tile([P, T], F32)
        exs = pp.tile([P, T], F32)       # exp(scores), filled per segment
        part = pp.tile([P, T // BTT], F32)  # per-segment exp sums

        enct = {}

        def load_tile(n):
            """SWDGE cast-DMA: pre-tiled encT bf16 -> fp8, one [128, 2, 512]
            tile per DoubleRow group so each group's matmuls can start as
            soon as its own quarter has landed."""
            quarters = []
            for g in range(NG):
                t_ = ep.tile([P, 2, BTT], FP8, tag="enc", name=f"enc{n}_{g}")
                nc.gpsimd.dma_start(
                    out=t_, in_=ENCT[n, :, 2 * g:2 * g + 2, :])
                quarters.append(t_)
            return quarters

        # ---- softmax over T for one batch elem (scores row 32*bi) ----
        def softmax_row(bi):
            # rows 0-2 have no downstream consumers, so the list scheduler
            # would otherwise defer them into the kernel tail
            if bi < NB - 1:
                with tc.high_priority():
                    _softmax_row(bi)
            else:
                _softmax_row(bi)

        def _softmax_row(bi):
            ssum = smp.tile([1, 1], F32, tag="ssum", name=f"ssum{bi}", bufs=NB)
            nc.vector.reduce_sum(ssum, part[32 * bi:32 * bi + 1, :],
                                 axis=mybir.AxisListType.X)
            rinv = smp.tile([1, 1], F32, tag="rinv", name=f"rinv{bi}", bufs=NB)
            nc.vector.reciprocal(rinv, ssum)
            o_sb = smp.tile([1, T], F32, tag="osb", name=f"osb{bi}", bufs=2)
            nc.vector.tensor_scalar_mul(o_sb, exs[32 * bi:32 * bi + 1, :],
                                        rinv[:, 0:1])
            nc.sync.dma_start(out=OUT[bi:bi + 1, :], in_=o_sb)

        def vdot(ps_s, e_bf, di):
            jj = di % 4
            nc.tensor.matmul(
                ps_s[32 * jj:32 * jj + 1, :], vt[:, di:di + 1], e_bf,
                start=(di < 4), stop=(di >= 4),
                tile_position=(0, 32 * jj))

        # ---- cross-position reduction + exp + (maybe) softmax ----
        def drain_scores(pend):
            ps_s, bi, toff = pend
            # PSUM has 1 DVE read port -> stage via SBUF
            sacc = smp.tile([1, BTT], F32, tag="sacc", name=f"sacc{toff}_{bi}",
                            bufs=2)
            nc.scalar.copy(sacc, ps_s[0:1, :])
            nc.vector.tensor_add(sacc, sacc, ps_s[32:33, :])
            nc.vector.tensor_add(sacc, sacc, ps_s[64:65, :])
            nc.vector.tensor_add(
                scores[32 * bi:32 * bi + 1, toff:toff + BTT],
                sacc, ps_s[96:97, :])
            seg = toff // BTT
            nc.scalar.activation(
                exs[32 * bi:32 * bi + 1, toff:toff + BTT],
                scores[32 * bi:32 * bi + 1, toff:toff + BTT], EXP,
                bias=0.0, scale=1.0,
                accum_out=part[32 * bi:32 * bi + 1, seg:seg + 1])
            if toff == T - BTT:
                softmax_row(bi)

        # tiles 0-1 ship pre-cast fp8 and load first, interleaved with the
        # w2 quarters, so the first DoubleRow group starts as early as the
        # DMA path allows
        enct[0] = [ep.tile([P, 2, BTT], FP8, tag="enc", name=f"enc0_{g}")
                   for g in range(NG)]
        enct[1] = [ep.tile([P, 2, BTT], FP8, tag="enc", name=f"enc1_{g}")
                   for g in range(NG)]
        # SWDGE spreads each transfer over all 16 DMA engines; the HWDGE
        # rings are single-engine (~27 GB/s) and would gate the first matmul
        nc.gpsimd.dma_start(out=enct[0][0], in_=ENC0[0, :, 0:2, :])
        nc.gpsimd.dma_start(out=w2d[0], in_=W2R[:, 0])
        nc.gpsimd.dma_start(out=enct[0][1], in_=ENC0[0, :, 2:4, :])
        nc.gpsimd.dma_start(out=w2d[1], in_=W2R[:, 1])
        nc.gpsimd.dma_start(out=enct[0][2], in_=ENC0[0, :, 4:6, :])
        nc.gpsimd.dma_start(out=w2d[2], in_=W2R[:, 2])
        nc.gpsimd.dma_start(out=enct[0][3], in_=ENC0[0, :, 6:8, :])
        nc.gpsimd.dma_start(out=w2d[3], in_=W2R[:, 3])
        for di in range(4, DT):
            nc.gpsimd.dma_start(out=w2d[di], in_=W2R[:, di])
        nc.sync.dma_start(out=hb, in_=HB)
        nc.sync.dma_start(out=vt, in_=VT)
        for g in range(NG):
            nc.gpsimd.dma_start(out=enct[1][g], in_=ENC0[1, :, 2 * g:2 * g + 2, :])
        # prefetch bt-tiles 2-5 through the SWDGE cast stream
        for n in range(2, 6):
            enct[n] = load_tile(n)

        # ---- main loop over bt-tiles ----
        pend = None          # (ps_s, e_list, bi, toff) awaiting v-dot matmuls
        for n in range(N_BT):
            bi = n // (T // BTT)
            toff = (n % (T // BTT)) * BTT
            last = n >= N_BT - 2
            if n + 6 < N_BT:
                enct[n + 6] = load_tile(n + 6)
            tiles = enct.pop(n)
            if n == N_BT - 1:  # tail: psum banks free as main loop winds down
                ps_s = pep.tile([P, BTT], F32, tag="e", name=f"ps_s{n}")
            else:
                ps_s = psp.tile([P, BTT], F32, tag="s", name=f"ps_s{n}")
            e_list = []
            for di in range(DT):
                ps_e = pep.tile([P, BTT], F32, tag="e", name=f"ps_e{n}_{di}")
                for g in range(NG):
                    nc.tensor.matmul(
                        ps_e, w2g(g, di), tiles[g],
                        start=(g == 0), stop=(g == NG - 1),
                        perf_mode=DR)
                # flush the previous tile's v-dots once its relus are long
                # done (di==5): the 8 matmuls stay adjacent -> 4-wide packing
                if di == 5 and pend is not None:
                    for dj in range(DT):
                        vdot(pend[0], pend[1][dj], dj)
                    drain_scores((pend[0], pend[2], pend[3]))
                    pend = None
                e_bf = ebp.tile([P, BTT], BF16, tag="eb", name=f"e{n}_{di}")
                col = hb[:, di * NB + bi:di * NB + bi + 1]
                on_dve = (di % 2 == 1) if n == N_BT - 1 else (di in (3, 7))
                if on_dve:   # keep ACT ahead of PSUM release / shorten tail
                    nc.vector.tensor_scalar(e_bf, ps_e, col, 0.0,
                                            op0=ADD, op1=MAX)
                else:
                    nc.scalar.activation(e_bf, ps_e, RELU, bias=col, scale=1.0)
                e_list.append(e_bf)
                if last:   # tail: v-dot each d-tile as soon as its relu lands
                    nc.tensor.matmul(
                        ps_s[0:1, :], vt[:, di:di + 1], e_bf,
                        start=(di == 0), stop=(di == DT - 1),
                        tile_position=(0, 0))
            if last:
                # exp straight off the psum score row; skip the 4-way drain
                nc.scalar.activation(
                    exs[32 * bi:32 * bi + 1, toff:toff + BTT],
                    ps_s[0:1, :], EXP, bias=0.0, scale=1.0,
                    accum_out=part[32 * bi:32 * bi + 1,
                                   toff // BTT:toff // BTT + 1])
                if toff == T - BTT:
                    softmax_row(bi)
            else:
                pend = (ps_s, e_list, bi, toff)

    nc.compile()
    return nc


def make_in_maps(hidden, enc, W, b, v):
    """Per-core input dicts: batch-sharded encT, replicated small tensors.
    encT is the per-core enc slice transposed to [D, bt] (contraction dim on
    SBUF partitions -> no device transposes). W2.T ships as fp8e4 scaled by
    W_SCALE to dodge e4m3 subnormals (undone via v/W_SCALE); hb folds the
    hidden/bias half of the affine into the relu bias (scaled to match)."""
    W1, W2 = W[:, :D], W[:, D:]
    hb_all = ((hidden @ W1.T + b) * W_SCALE).astype(np.float32)   # [B, D]
    w2q = np.ascontiguousarray(W2.T * W_SCALE).astype(ml_dtypes.float8_e4m3)
    w2q = w2q.reshape(KT, P, DT, P).transpose(1, 2, 0, 3).reshape(P, KT * D)
    vt = np.asarray(v, np.float32).reshape(DT, P).T / W_SCALE
    vt = np.ascontiguousarray(vt).astype(ml_dtypes.bfloat16)
    maps = []
    for c in range(N_CORES):
        enc_c = enc[c * NB:(c + 1) * NB].reshape(BT, D)
        # pre-tiled transpose: enct[n, p, kj, c] = enc_c[n*512+c, kj*128+p]
        enct = np.ascontiguousarray(
            enc_c.reshape(N_BT, BTT, KT, P).transpose(0, 3, 2, 1)).astype(
                ml_dtypes.bfloat16)
        hb_c = hb_all[c * NB:(c + 1) * NB]                    # [NB, D]
        hb_dev = np.ascontiguousarray(
            hb_c.T.reshape(DT, P, NB).transpose(1, 0, 2).reshape(P, DT * NB))
        enc0 = enct[:2].astype(ml_dtypes.float8_e4m3)
        maps.append(dict(enct=enct, enc0=enc0, w2q=w2q, hb=hb_dev, vt=vt))
    return maps


_NC_CACHE = []


def kernel(hidden, encoder_outputs, W, b, v):
    hidden = np.asarray(hidden, dtype=np.float32)
    enc = np.asarray(encoder_outputs, dtype=np.float32)
    W = np.asarray(W, dtype=np.float32)
    b = np.asarray(b, dtype=np.float32)
    v = np.asarray(v, dtype=np.float32)

    if not _NC_CACHE:
        _NC_CACHE.append(_build())
    nc = _NC_CACHE[0]

    in_maps = make_in_maps(hidden, enc, W, b, v)
    res = bass_utils.run_bass_kernel_spmd(nc, in_maps, core_ids=list(range(N_CORES)))
    scores = np.concatenate([res.results[c]["out"] for c in range(N_CORES)], axis=0)
    return scores[:, None, :].astype(np.float32)


# revision 20
# speedup vs baseline: 1.1848x; 1.1848x over previous
"""Bahdanau-attention kernel for Trainium2 (8 NeuronCores, data-parallel over batch).

reference math:
  energy = relu(concat([hidden bcast T, enc], -1) @ W.T + b)   # [B,T,D]
  scores = energy @ v                                          # [B,T]
  out    = softmax(scores, axis=T)[:, None, :]                 # [B,1,T]

Per-core kernel (4 batch elems, 8192 bt rows):
  W = [W1 | W2]; hb = (hid @ W1.T + b) * S is computed on host (tiny) and
  folded into the relu bias; W2.T ships pre-scaled (x S=8192) in fp8e4
  DoubleRow layout (the scale dodges e4m3 subnormals and is undone by
  shipping v/S). enc ships pre-transposed + pre-tiled (encT bf16, tiles 0-1
  pre-cast fp8) so the device does zero PE transposes: SWDGE cast-DMA loads
  encT straight into fp8 [128, 2, 512] quarter-tiles (8 KB-contiguous
  descriptor runs), the main matmul runs fp8 DoubleRow (K=256 per MM, 512
  MMs at the 213 ns N=512 streaming floor), relu(psum + hb*S) -> bf16 runs
  on ACT with d-tiles 3/7 on DVE (keeps PSUM banks recycling ahead of the
  PE), and the v-dot contracts d via 4-wide col-group-packed bf16 PE matmuls
  (tile_position) flushed one tile late so all relu deps are met, with
  cross-position DVE adds and fp32 softmax over T per batch elem. The last
  two bt-tiles instead chain single-position v-dots and exp() straight off
  the PSUM score row so the kernel tail is just one softmax. Junk warmup
  matmuls at t=0 cover the first DMA and hold the PE HAM clock-gate at 8/8.
"""
import numpy as np
import ml_dtypes
import concourse.mybir as mybir
import concourse.tile as tile
import concourse.bacc as bacc
from concourse import bass_utils

P = 128
B, T, D = 32, 2048, 1024
N_CORES = 8
NB = B // N_CORES            # 4 local batch elems
BT = NB * T                  # 8192 local rows
BTT = 512                    # bt-tile (columns of energy^T)
N_BT = BT // BTT             # 16 bt-tiles
DT = D // P                  # 8 d-tiles (output dim of W2)
KT = D // P                  # 8 k-subtiles (contraction over enc features)
NG = KT // 2                 # 4 DoubleRow groups (K=256 each)
W_SCALE = 8192.0             # keeps fp8(W2*S) in e4m3 normal range
BF16, F32 = mybir.dt.bfloat16, mybir.dt.float32
FP8 = mybir.dt.float8e4
RELU = mybir.ActivationFunctionType.Relu
EXP = mybir.ActivationFunctionType.Exp
DR = mybir.MatmulPerfMode.DoubleRow
ADD, MAX = mybir.AluOpType.add, mybir.AluOpType.max


def _build():
    nc = bacc.Bacc("TRN2", target_bir_lowering=False, debug=False)
    ENCT = nc.dram_tensor("enct", [N_BT, P, KT, BTT], BF16, kind="ExternalInput").ap()
    ENC0 = nc.dram_tensor("enc0", [2, P, KT, BTT], FP8, kind="ExternalInput").ap()
    W2Q = nc.dram_tensor("w2q", [P, KT * D], FP8, kind="ExternalInput").ap()
    HB = nc.dram_tensor("hb", [P, DT * NB], F32, kind="ExternalInput").ap()
    VT = nc.dram_tensor("vt", [P, DT], BF16, kind="ExternalInput").ap()
    OUT = nc.dram_tensor("out", [NB, T], F32, kind="ExternalOutput").ap()

    with tile.TileContext(nc) as tc, \
         tc.tile_pool(name="persist", bufs=1) as pp, \
         tc.tile_pool(name="enc_sb", bufs=32) as ep, \
         tc.tile_pool(name="e_sb", bufs=24) as ebp, \
         tc.tile_pool(name="ps_e", bufs=6, space="PSUM") as pep, \
         tc.tile_pool(name="ps_s", bufs=2, space="PSUM") as psp, \
         tc.tile_pool(name="sm", bufs=1) as smp:

        # ---- PE warmup: junk matmuls cover the initial DMA wait and get the
        # HAM clock gate to 8/8 before the first real matmul ----
        junk = pp.tile([P, P], BF16)
        nc.vector.memset(junk, 0.0)
        jps = pep.tile([P, P], F32, tag="e", name="junk_ps")
        for _ in range(52):
            nc.tensor.matmul(jps, junk, junk, start=True, stop=True)

        # persistent: DoubleRow-layout W2 quarters, fused hidden/bias, v/S
        w2q_ = [pp.tile([P, 2, D], FP8, name=f"w2_{g}") for g in range(NG)]
        hb = pp.tile([P, DT * NB], F32)  # col di*NB+b = ((hid@W1.T)[b,d]+bias)*S
        vt = pp.tile([P, DT], BF16)      # col di = v[di*128 : (di+1)*128] / S
        W2R = W2Q.rearrange("p (kj d) -> p kj d", kj=KT)

        def w2g(g, di):
            return w2q_[g][:, :, di * P:(di + 1) * P]

        # batch elem bi lives on partition 32*bi (compute outputs need
        # 32-aligned partition bases)
        scores = pp.tile([P, T], F32)
        exs = pp.tile([P, T], F32)       # exp(scores), filled per segment
        part = pp.tile([P, T // BTT], F32)  # per-segment exp sums

        enct = {}

        def load_tile(n):
            """SWDGE cast-DMA: pre-tiled encT bf16 -> fp8, one [128, 2, 512]
            tile per DoubleRow group so each group's matmuls can start as
            soon as its own quarter has landed."""
            quarters = []
            for g in range(NG):
                t_ = ep.tile([P, 2, BTT], FP8, tag="enc", name=f"enc{n}_{g}")
                nc.gpsimd.dma_start(
                    out=t_, in_=ENCT[n, :, 2 * g:2 * g + 2, :])
                quarters.append(t_)
            return quarters

        # ---- softmax over T for one batch elem (scores row 32*bi) ----
        def softmax_row(bi):
            # rows 0-2 have no downstream consumers, so the list scheduler
            # would otherwise defer them into the kernel tail
            if bi < NB - 1:
                with tc.high_priority():
                    _softmax_row(bi)
            else:
                _softmax_row(bi)

        def _softmax_row(bi):
            ssum = smp.tile([1, 1], F32, tag="ssum", name=f"ssum{bi}", bufs=NB)
            nc.vector.reduce_sum(ssum, part[32 * bi:32 * bi + 1, :],
                                 axis=mybir.AxisListType.X)
            rinv = smp.tile([1, 1], F32, tag="rinv", name=f"rinv{bi}", bufs=NB)
            nc.vector.reciprocal(rinv, ssum)
            o_sb = smp.tile([1, T], F32, tag="osb", name=f"osb{bi}", bufs=2)
            nc.vector.tensor_scalar_mul(o_sb, exs[32 * bi:32 * bi + 1, :],
                                        rinv[:, 0:1])
            nc.sync.dma_start(out=OUT[bi:bi + 1, :], in_=o_sb)

        def vdot(ps_s, e_bf, di):
            jj = di % 4
            nc.tensor.matmul(
                ps_s[32 * jj:32 * jj + 1, :], vt[:, di:di + 1], e_bf,
                start=(di < 4), stop=(di >= 4),
                tile_position=(0, 32 * jj))

        # ---- cross-position reduction + exp + (maybe) softmax ----
        def drain_scores(pend):
            ps_s, bi, toff = pend
            # PSUM has 1 DVE read port -> stage via SBUF
            sacc = smp.tile([1, BTT], F32, tag="sacc", name=f"sacc{toff}_{bi}",
                            bufs=2)
            nc.scalar.copy(sacc, ps_s[0:1, :])
            nc.vector.tensor_add(sacc, sacc, ps_s[32:33, :])
            nc.vector.tensor_add(sacc, sacc, ps_s[64:65, :])
            nc.vector.tensor_add(
                scores[32 * bi:32 * bi + 1, toff:toff + BTT],
                sacc, ps_s[96:97, :])
            seg = toff // BTT
            nc.scalar.activation(
                exs[32 * bi:32 * bi + 1, toff:toff + BTT],
                scores[32 * bi:32 * bi + 1, toff:toff + BTT], EXP,
                bias=0.0, scale=1.0,
                accum_out=part[32 * bi:32 * bi + 1, seg:seg + 1])
            if toff == T - BTT:
                softmax_row(bi)

        # tiles 0-1 ship pre-cast fp8 and load first, interleaved with the
        # w2 quarters, so the first DoubleRow group starts as early as the
        # DMA path allows
        enct[0] = [ep.tile([P, 2, BTT], FP8, tag="enc", name=f"enc0_{g}")
                   for g in range(NG)]
        enct[1] = [ep.tile([P, 2, BTT], FP8, tag="enc", name=f"enc1_{g}")
                   for g in range(NG)]
        # SWDGE spreads each transfer over all 16 DMA engines; the HWDGE
        # rings are single-engine (~27 GB/s) and would gate the first matmul
        nc.gpsimd.dma_start(out=enct[0][0], in_=ENC0[0, :, 0:2, :])
        nc.gpsimd.dma_start(out=w2q_[0], in_=W2R[:, 0:2, :])
        nc.gpsimd.dma_start(out=enct[0][1], in_=ENC0[0, :, 2:4, :])
        nc.gpsimd.dma_start(out=w2q_[1], in_=W2R[:, 2:4, :])
        nc.gpsimd.dma_start(out=enct[0][2], in_=ENC0[0, :, 4:6, :])
        nc.gpsimd.dma_start(out=w2q_[2], in_=W2R[:, 4:6, :])
        nc.gpsimd.dma_start(out=enct[0][3], in_=ENC0[0, :, 6:8, :])
        nc.gpsimd.dma_start(out=w2q_[3], in_=W2R[:, 6:8, :])
        nc.sync.dma_start(out=hb, in_=HB)
        nc.sync.dma_start(out=vt, in_=VT)
        for g in range(NG):
            nc.gpsimd.dma_start(out=enct[1][g], in_=ENC0[1, :, 2 * g:2 * g + 2, :])
        # prefetch bt-tiles 2-5 through the SWDGE cast stream
        for n in range(2, 6):
            enct[n] = load_tile(n)

        # ---- main loop over bt-tiles ----
        pend = None          # (ps_s, e_list, bi, toff) awaiting v-dot matmuls
        for n in range(N_BT):
            bi = n // (T // BTT)
            toff = (n % (T // BTT)) * BTT
            last = n >= N_BT - 2
            if n + 6 < N_BT:
                enct[n + 6] = load_tile(n + 6)
            tiles = enct.pop(n)
            if n == N_BT - 1:  # tail: psum banks free as main loop winds down
                ps_s = pep.tile([P, BTT], F32, tag="e", name=f"ps_s{n}")
            else:
                ps_s = psp.tile([P, BTT], F32, tag="s", name=f"ps_s{n}")
            e_list = []
            for di in range(DT):
                ps_e = pep.tile([P, BTT], F32, tag="e", name=f"ps_e{n}_{di}")
                for g in range(NG):
                    nc.tensor.matmul(
                        ps_e, w2g(g, di), tiles[g],
                        start=(g == 0), stop=(g == NG - 1),
                        perf_mode=DR)
                # flush the previous tile's v-dots once its relus are long
                # done (di==5): the 8 matmuls stay adjacent -> 4-wide packing
                if di == 5 and pend is not None:
                    for dj in range(DT):
                        vdot(pend[0], pend[1][dj], dj)
                    drain_scores((pend[0], pend[2], pend[3]))
                    pend = None
                e_bf = ebp.tile([P, BTT], BF16, tag="eb", name=f"e{n}_{di}")
                col = hb[:, di * NB + bi:di * NB + bi + 1]
                on_dve = (di % 2 == 1) if n == N_BT - 1 else (di in (3, 7))
                if on_dve:   # keep ACT ahead of PSUM release / shorten tail
                    nc.vector.tensor_scalar(e_bf, ps_e, col, 0.0,
                                            op0=ADD, op1=MAX)
                else:
                    nc.scalar.activation(e_bf, ps_e, RELU, bias=col, scale=1.0)
                e_list.append(e_bf)
                if last:   # tail: v-dot each d-tile as soon as its relu lands
                    nc.tensor.matmul(
                        ps_s[0:1, :], vt[:, di:di + 1], e_bf,
                        start=(di == 0), stop=(di == DT - 1),
                        tile_position=(0, 0))
            if last:
                # exp straight off the psum score row; skip the 4-way drain
                nc.scalar.activation(
                    exs[32 * bi:32 * bi + 1, toff:toff + BTT],
                    ps_s[0:1, :], EXP, bias=0.0, scale=1.0,
                    accum_out=part[32 * bi:32 * bi + 1,
                                   toff // BTT:toff // BTT + 1])
                if toff == T - BTT:
                    softmax_row(bi)
            else:
                pend = (ps_s, e_list, bi, toff)

    nc.compile()
    return nc


def make_in_maps(hidden, enc, W, b, v):
    """Per-core input dicts: batch-sharded encT, replicated small tensors.
    encT is the per-core enc slice transposed to [D, bt] (contraction dim on
    SBUF partitions -> no device transposes). W2.T ships as fp8e4 scaled by
    W_SCALE to dodge e4m3 subnormals (undone via v/W_SCALE); hb folds the
    hidden/bias half of the affine into the relu bias (scaled to match)."""
    W1, W2 = W[:, :D], W[:, D:]
    hb_all = ((hidden @ W1.T + b) * W_SCALE).astype(np.float32)   # [B, D]
    w2q = np.ascontiguousarray(W2.T * W_SCALE).astype(ml_dtypes.float8_e4m3)
    w2q = w2q.reshape(KT, P, D).transpose(1, 0, 2).reshape(P, KT * D)
    vt = np.asarray(v, np.float32).reshape(DT, P).T / W_SCALE
    vt = np.ascontiguousarray(vt).astype(ml_dtypes.bfloat16)
    maps = []
    for c in range(N_CORES):
        enc_c = enc[c * NB:(c + 1) * NB].reshape(BT, D)
        # pre-tiled transpose: enct[n, p, kj, c] = enc_c[n*512+c, kj*128+p]
        enct = np.ascontiguousarray(
            enc_c.reshape(N_BT, BTT, KT, P).transpose(0, 3, 2, 1)).astype(
                ml_dtypes.bfloat16)
        hb_c = hb_all[c * NB:(c + 1) * NB]                    # [NB, D]
        hb_dev = np.ascontiguousarray(
            hb_c.T.reshape(DT, P, NB).transpose(1, 0, 2).reshape(P, DT * NB))
        enc0 = enct[:2].astype(ml_dtypes.float8_e4m3)
        maps.append(dict(enct=enct, enc0=enc0, w2q=w2q, hb=hb_dev, vt=vt))
    return maps


_NC_CACHE = []


def kernel(hidden, encoder_outputs, W, b, v):
    hidden = np.asarray(hidden, dtype=np.float32)
    enc = np.asarray(encoder_outputs, dtype=np.float32)
    W = np.asarray(W, dtype=np.float32)
    b = np.asarray(b, dtype=np.float32)
    v = np.asarray(v, dtype=np.float32)

    if not _NC_CACHE:
        _NC_CACHE.append(_build())
    nc = _NC_CACHE[0]

    in_maps = make_in_maps(hidden, enc, W, b, v)
    res = bass_utils.run_bass_kernel_spmd(nc, in_maps, core_ids=list(range(N_CORES)))
    scores = np.concatenate([res.results[c]["out"] for c in range(N_CORES)], axis=0)
    return scores[:, None, :].astype(np.float32)
